# revision 12
# baseline (speedup 1.0000x reference)
"""Bass/Trainium2 kernel for nn_DecodeWrapperEager (sparse paged-attention decode).

Problem (hardcoded): B=8, Hq=32, Hk=8, D=64, S=32, NB=128, T=4096, W=1024.
One decode step of GQA attention with a paged KV cache, sliding window W and
a sink logit.  Output: [B, 1, Hq, D] float32.

Sharding: KV heads across the 8 cores (tensor-parallel).  Each core receives
its head's cache in two layouts staged on host (bf16 for matmul speed):
  - ktc [64, 32768]  : K^T, column = page*32 + slot (page-linear), k_last
                       scattered at the decode position.
  - vc  [32768, 65]  : V rows page-linear, with a fused ones-column (col 64)
                       so the P @ [V|1] matmul yields the softmax denominator.
Per-batch sliding-window offsets are identical on every core, so they are
baked into the single SPMD program; the device gathers only the window pages.
Batches are packed in pairs onto the 128-partition contraction dim (head dim
is 64) so the PE array and all 16 DMA ports are fully used.  Out-of-window
tail positions are handled by zero-filling K columns (score 0 -> weight 1)
and V rows (weight * 0 = 0 contribution), so every matmul is full-size.
"""

import os
import numpy as np
import ml_dtypes

import concourse.bacc as bacc
import concourse.tile as tile
import concourse.mybir as mybir
from concourse.bass_utils import run_bass_kernel_spmd

F32 = mybir.dt.float32
F32R = mybir.dt.float32r   # single-pass PE mode, ~1.6e-4 rel (vs 3e-3 bf16)
BF16 = mybir.dt.bfloat16
BF16_NP = ml_dtypes.bfloat16

B, Hq, Hk, D = 8, 32, 8, 64
S, NB = 32, 128
T = NB * S          # 4096 positions per sequence
P = B * NB          # 1024 cache pages
W = 1024            # sliding window
G = Hq // Hk        # 4 query heads per kv head
NCORES = 8
PL = P * S          # 32768 page-linear positions
DV = D + 1          # V row with fused ones-column

LAST_EXEC_NS = None


def _window_runs(sbi_b, pos):
    """Contiguous runs (col, len) in page-linear space covering [lo, pos]."""
    lo = max(0, pos - W + 1)
    nv = pos + 1 - lo
    runs = []
    t = lo
    while t <= pos:
        blk = t // S
        s0 = t % S
        s1 = min(S - 1, pos - blk * S)
        col = int(sbi_b[blk]) * S + s0
        ln = s1 - s0 + 1
        if runs and runs[-1][0] + runs[-1][1] == col:
            runs[-1][1] += ln
        else:
            runs.append([col, ln])
        t = blk * S + s1 + 1
    return lo, nv, [(c, l) for c, l in runs]


def _pairs_of(win):
    order = sorted(range(B), key=lambda b: -win[b][1])
    return [(order[i], order[i + 1]) for i in range(0, B, 2)]


def _build_program(win):
    """win: list per batch of (lo, nv, runs). Returns finalized Bacc program.

    The program is shared by all 8 cores (SPMD); only the data differs.
    """
    nc = bacc.Bacc("TRN2", debug=False)

    ktc = nc.dram_tensor("ktc", [D, PL], F32R, kind="ExternalInput")
    vc = nc.dram_tensor("vc", [PL, DV], BF16, kind="ExternalInput")
    qtz = nc.dram_tensor("qtz", [128, B * G], F32R, kind="ExternalInput")
    sk = nc.dram_tensor("sk", [G, 1], F32, kind="ExternalInput")
    out = nc.dram_tensor("out", [B, G, D], F32, kind="ExternalOutput")

    pairs = _pairs_of(win)
    nch = {b: (win[b][1] + 127) // 128 for b in range(B)}
    rem = {b: win[b][1] - (nch[b] - 1) * 128 for b in range(B)}

    # V DMAs alternate between otherwise-idle queues to parallelize
    # descriptor generation; K/out/consts go on the sync HWDGE ring.
    vdma_engines = [None, None]

    with tile.TileContext(nc) as tc:
        vdma_engines = [nc.gpsimd, nc.scalar]
        with (
            tc.tile_pool(name="const", bufs=1) as cpool,
            tc.tile_pool(name="kt", bufs=3) as kpool,
            tc.tile_pool(name="vv", bufs=3) as vpool,
            tc.tile_pool(name="ee", bufs=3) as epool,
            tc.tile_pool(name="small", bufs=4) as spool,
            tc.tile_pool(name="outp", bufs=4) as opool,
            tc.tile_pool(name="qkp", bufs=3, space="PSUM") as qkpool,
            tc.tile_pool(name="pvp", bufs=3, space="PSUM") as pvpool,
        ):
            qtz_sb = cpool.tile([128, B * G], F32R)
            nc.sync.dma_start(qtz_sb[:], qtz[:])
            sk_sb = cpool.tile([G, 1], F32)
            nc.sync.dma_start(sk_sb[:], sk[:])
            esink = cpool.tile([G, 1], F32)
            nc.scalar.activation(esink[:], sk_sb[:], mybir.ActivationFunctionType.Exp)

            for pi, (bA, bB) in enumerate(pairs):
                ncp = max(nch[bA], nch[bB])
                ktile = kpool.tile([128, ncp * 128], F32R)
                # extra DV columns of slack so the strided [p, c, 0:DV]
                # multi-chunk DMA view for half 1 stays in bounds
                vt = vpool.tile([128, ncp * 2 * DV + DV], BF16)
                for half, b in ((0, bA), (1, bB)):
                    lo, nv, runs = win[b]
                    veng = vdma_engines[half]
                    # K^T window -> rows [half*64, half*64+64)
                    o = 0
                    for col, ln in runs:
                        nc.sync.dma_start(
                            ktile[half * 64:half * 64 + 64, o:o + ln],
                            ktc[:, col:col + ln],
                        )
                        o += ln
                    if nv < ncp * 128:
                        # zero-fill stale K cols: score 0 -> e = 1, killed by
                        # the zero-filled V rows below
                        nc.gpsimd.memset(
                            ktile[half * 64:half * 64 + 64,
                                  nv:ncp * 128].bitcast(F32), 0.0
                        )
                    # zero-fill V beyond the window first (memset needs
                    # 32-aligned partition starts, so cover all 128 rows and
                    # let the window DMAs below overwrite the valid rows)
                    if rem[b] < 128:
                        cbase = (nch[b] - 1) * 2 * DV + half * DV
                        nc.gpsimd.memset(vt[:, cbase:cbase + DV], 0.0)
                    for c in range(nch[b], ncp):
                        cbase = c * 2 * DV + half * DV
                        nc.gpsimd.memset(vt[:, cbase:cbase + DV], 0.0)
                    # V window chunks -> cols [c*130 + half*65, +65)
                    o = 0
                    for col, ln in runs:
                        while ln > 0:
                            c, p0 = divmod(o, 128)
                            cbase = c * 2 * DV + half * DV
                            if p0 == 0 and ln >= 128:
                                nf = ln // 128
                                if nf > 1:
                                    # [p, c, 0:65] strided view over chunk slots
                                    dst = vt[:, cbase:cbase + nf * 2 * DV]
                                    dst = dst.rearrange(
                                        "p (c f) -> p c f", f=2 * DV)[:, :, 0:DV]
                                    src = vc[col:col + nf * 128, :].rearrange(
                                        "(c p) f -> p c f", p=128)
                                else:
                                    dst = vt[:, cbase:cbase + DV]
                                    src = vc[col:col + 128, :]
                                veng.dma_start(dst, src)
                                adv = nf * 128
                            else:
                                seg = min(128 - p0, ln)
                                veng.dma_start(
                                    vt[p0:p0 + seg, cbase:cbase + DV],
                                    vc[col:col + seg, :],
                                )
                                adv = seg
                            o += adv
                            col += adv
                            ln -= adv
                qk = qkpool.tile([128, ncp * 2 * G], F32)
                for c in range(ncp):
                    nc.tensor.matmul(
                        qk[:, c * 2 * G:(c + 1) * 2 * G],
                        ktile[:, c * 128:(c + 1) * 128],
                        qtz_sb[:, pi * 2 * G:(pi + 1) * 2 * G],
                        start=True, stop=True, skip_group_check=True,
                    )
                e = epool.tile([128, ncp * 2 * G], BF16)
                nc.scalar.activation(
                    e[:], qk[:], mybir.ActivationFunctionType.Exp,
                    scale=1.0 / 8.0,
                )
                pv = pvpool.tile([2 * G, 2 * DV], F32)
                for c in range(ncp):
                    nc.tensor.matmul(
                        pv[:],
                        e[:, c * 2 * G:(c + 1) * 2 * G],
                        vt[:, c * 2 * DV:(c + 1) * 2 * DV],
                        start=(c == 0), stop=(c == ncp - 1),
                    )

                # compute engines need 32-aligned partition starts, so copy
                # pv to SBUF and DMA-shift batch B's quadrant to partition 0
                pvsb = spool.tile([2 * G, 2 * DV], F32, tag="pvsb")
                nc.vector.tensor_copy(pvsb[:], pv[:])
                pvb = spool.tile([G, DV], F32, tag="pvb")
                nc.sync.dma_start(pvb[:], pvsb[G:2 * G, DV:2 * DV])
                for half, b in ((0, bA), (1, bB)):
                    pvh = pvsb[0:G, 0:DV] if half == 0 else pvb[:]
                    den = spool.tile([G, 1], F32, tag="den")
                    nc.vector.tensor_add(den[:], pvh[:, D:DV], esink[:])
                    inv = spool.tile([G, 1], F32, tag="inv")
                    nc.vector.reciprocal(inv[:], den[:])
                    osb = opool.tile([G, D], F32)
                    nc.scalar.mul(osb[:], pvh[:, 0:D], inv[:])
                    nc.sync.dma_start(out[b], osb[:])

    nc.finalize()
    return nc


_PROGRAM_CACHE = {}


def kernel(q_last, k_last, v_last, cache_state, seq_block_ids, start_positions,
           sink):
    global LAST_EXEC_NS

    q_last = np.asarray(q_last, dtype=np.float32)
    k_last = np.asarray(k_last, dtype=np.float32)
    v_last = np.asarray(v_last, dtype=np.float32)
    cache_state = np.asarray(cache_state, dtype=np.float32)
    sbi = np.asarray(seq_block_ids)
    pos = np.asarray(start_positions)
    sink = np.asarray(sink, dtype=np.float32)

    win = [_window_runs(sbi[b], int(pos[b])) for b in range(B)]

    key = (tuple(int(p) for p in pos), sbi.tobytes())
    if key not in _PROGRAM_CACHE:
        _PROGRAM_CACHE.clear()
        _PROGRAM_CACHE[key] = _build_program(win)
    nc = _PROGRAM_CACHE[key]

    pairs = _pairs_of(win)

    # decode-token scatter targets (page-linear)
    scat = [int(sbi[b, int(pos[b]) // S]) * S + int(pos[b]) % S for b in range(B)]

    in_maps = []
    for h in range(Hk):
        kt = np.ascontiguousarray(cache_state[:, 0, h].reshape(PL, D).T)
        vcv = np.empty((PL, DV), dtype=BF16_NP)
        vcv[:, :D] = cache_state[:, 1, h].reshape(PL, D).astype(BF16_NP)
        vcv[:, D] = 1.0
        for b in range(B):
            kt[:, scat[b]] = k_last[b, 0, h]
            vcv[scat[b], :D] = v_last[b, 0, h].astype(BF16_NP)
        qz = np.zeros((128, B * G), dtype=np.float32)
        for pi, (bA, bB) in enumerate(pairs):
            qz[0:D, pi * 2 * G:pi * 2 * G + G] = \
                q_last[bA, 0, h * G:(h + 1) * G].T
            qz[D:128, pi * 2 * G + G:(pi + 1) * 2 * G] = \
                q_last[bB, 0, h * G:(h + 1) * G].T
        skv = np.ascontiguousarray(sink[h * G:(h + 1) * G].reshape(G, 1))
        in_maps.append({"ktc": kt, "vc": vcv, "qtz": qz, "sk": skv})

    prof = os.environ.get("BASS_KERNEL_PROF", "") == "1"
    kwargs = {}
    if prof:
        kwargs = dict(trace=True, trace_cores=list(range(NCORES)))
        tdir = os.environ.get("BASS_KERNEL_PROF_DIR")
        if tdir:
            kwargs["tmpdir"] = tdir
    res = run_bass_kernel_spmd(nc, in_maps, list(range(NCORES)), **kwargs)
    if prof:
        LAST_EXEC_NS = res.exec_time_ns

    full = np.empty((B, 1, Hq, D), dtype=np.float32)
    for h in range(Hk):
        full[:, 0, h * G:(h + 1) * G, :] = res.results[h]["out"]
    return full


# revision 13
# speedup vs baseline: 1.3285x; 1.3285x over previous
"""Bass/Trainium2 kernel for nn_DecodeWrapperEager (sparse paged-attention decode).

Problem (hardcoded): B=8, Hq=32, Hk=8, D=64, S=32, NB=128, T=4096, W=1024.
One decode step of GQA attention with a paged KV cache, sliding window W and
a sink logit.  Output: [B, 1, Hq, D] float32.

Sharding: KV heads across the 8 cores (tensor-parallel).  Each core receives
its head's cache in two layouts staged on host (bf16 for matmul speed):
  - ktc [64, 32768]  : K^T, column = page*32 + slot (page-linear), k_last
                       scattered at the decode position.
  - vc  [32768, 65]  : V rows page-linear, with a fused ones-column (col 64)
                       so the P @ [V|1] matmul yields the softmax denominator.
Per-batch sliding-window offsets are identical on every core, so they are
baked into the single SPMD program; the device gathers only the window pages.
Batches are packed in pairs onto the 128-partition contraction dim (head dim
is 64) so the PE array and all 16 DMA ports are fully used.  Out-of-window
tail positions are handled by zero-filling K columns (score 0 -> weight 1)
and V rows (weight * 0 = 0 contribution), so every matmul is full-size.
"""

import os
import numpy as np

import concourse.bacc as bacc
import concourse.tile as tile
import concourse.mybir as mybir
from concourse.bass_utils import run_bass_kernel_spmd

F32 = mybir.dt.float32
F16 = mybir.dt.float16     # 10-bit mantissa: ~5e-4 end-to-end vs 3e-3 for bf16,
F16_NP = np.float16        # same PE speed (1 cycle/row) and DMA bytes as bf16

B, Hq, Hk, D = 8, 32, 8, 64
S, NB = 32, 128
T = NB * S          # 4096 positions per sequence
P = B * NB          # 1024 cache pages
W = 1024            # sliding window
G = Hq // Hk        # 4 query heads per kv head
NCORES = 8
PL = P * S          # 32768 page-linear positions
DV = D + 1          # V row with fused ones-column

LAST_EXEC_NS = None


def _window_runs(sbi_b, pos):
    """Contiguous runs (col, len) in page-linear space covering [lo, pos]."""
    lo = max(0, pos - W + 1)
    nv = pos + 1 - lo
    runs = []
    t = lo
    while t <= pos:
        blk = t // S
        s0 = t % S
        s1 = min(S - 1, pos - blk * S)
        col = int(sbi_b[blk]) * S + s0
        ln = s1 - s0 + 1
        if runs and runs[-1][0] + runs[-1][1] == col:
            runs[-1][1] += ln
        else:
            runs.append([col, ln])
        t = blk * S + s1 + 1
    return lo, nv, [(c, l) for c, l in runs]


def _pairs_of(win):
    order = sorted(range(B), key=lambda b: -win[b][1])
    return [(order[i], order[i + 1]) for i in range(0, B, 2)]


def _build_program(win):
    """win: list per batch of (lo, nv, runs). Returns finalized Bacc program.

    The program is shared by all 8 cores (SPMD); only the data differs.
    """
    nc = bacc.Bacc("TRN2", debug=False)

    ktc = nc.dram_tensor("ktc", [D, PL], F16, kind="ExternalInput")
    vc = nc.dram_tensor("vc", [PL, DV], F16, kind="ExternalInput")
    qtz = nc.dram_tensor("qtz", [128, B * G], F16, kind="ExternalInput")
    sk = nc.dram_tensor("sk", [G, 1], F32, kind="ExternalInput")
    out = nc.dram_tensor("out", [B, G, D], F32, kind="ExternalOutput")

    pairs = _pairs_of(win)
    nch = {b: (win[b][1] + 127) // 128 for b in range(B)}
    rem = {b: win[b][1] - (nch[b] - 1) * 128 for b in range(B)}

    # V DMAs alternate between otherwise-idle queues to parallelize
    # descriptor generation; K/out/consts go on the sync HWDGE ring.
    vdma_engines = [None, None]

    with tile.TileContext(nc) as tc:
        vdma_engines = [nc.gpsimd, nc.scalar]
        with (
            tc.tile_pool(name="const", bufs=1) as cpool,
            tc.tile_pool(name="kt", bufs=3) as kpool,
            tc.tile_pool(name="vv", bufs=3) as vpool,
            tc.tile_pool(name="ee", bufs=3) as epool,
            tc.tile_pool(name="small", bufs=4) as spool,
            tc.tile_pool(name="outp", bufs=4) as opool,
            tc.tile_pool(name="qkp", bufs=3, space="PSUM") as qkpool,
            tc.tile_pool(name="pvp", bufs=3, space="PSUM") as pvpool,
        ):
            qtz_sb = cpool.tile([128, B * G], F16)
            nc.sync.dma_start(qtz_sb[:], qtz[:])
            sk_sb = cpool.tile([G, 1], F32)
            nc.sync.dma_start(sk_sb[:], sk[:])
            esink = cpool.tile([G, 1], F32)
            nc.scalar.activation(esink[:], sk_sb[:], mybir.ActivationFunctionType.Exp)

            for pi, (bA, bB) in enumerate(pairs):
                ncp = max(nch[bA], nch[bB])
                ktile = kpool.tile([128, ncp * 128], F16)
                # extra DV columns of slack so the strided [p, c, 0:DV]
                # multi-chunk DMA view for half 1 stays in bounds
                vt = vpool.tile([128, ncp * 2 * DV + DV], F16)
                for half, b in ((0, bA), (1, bB)):
                    lo, nv, runs = win[b]
                    veng = vdma_engines[half]
                    # K^T window -> rows [half*64, half*64+64)
                    o = 0
                    for col, ln in runs:
                        nc.sync.dma_start(
                            ktile[half * 64:half * 64 + 64, o:o + ln],
                            ktc[:, col:col + ln],
                        )
                        o += ln
                    if nv < ncp * 128:
                        # zero-fill stale K cols: score 0 -> e = 1, killed by
                        # the zero-filled V rows below
                        nc.gpsimd.memset(
                            ktile[half * 64:half * 64 + 64, nv:ncp * 128], 0.0
                        )
                    # zero-fill V beyond the window first (memset needs
                    # 32-aligned partition starts, so cover all 128 rows and
                    # let the window DMAs below overwrite the valid rows)
                    if rem[b] < 128:
                        cbase = (nch[b] - 1) * 2 * DV + half * DV
                        nc.gpsimd.memset(vt[:, cbase:cbase + DV], 0.0)
                    for c in range(nch[b], ncp):
                        cbase = c * 2 * DV + half * DV
                        nc.gpsimd.memset(vt[:, cbase:cbase + DV], 0.0)
                    # V window chunks -> cols [c*130 + half*65, +65)
                    o = 0
                    for col, ln in runs:
                        while ln > 0:
                            c, p0 = divmod(o, 128)
                            cbase = c * 2 * DV + half * DV
                            if p0 == 0 and ln >= 128:
                                nf = ln // 128
                                if nf > 1:
                                    # [p, c, 0:65] strided view over chunk slots
                                    dst = vt[:, cbase:cbase + nf * 2 * DV]
                                    dst = dst.rearrange(
                                        "p (c f) -> p c f", f=2 * DV)[:, :, 0:DV]
                                    src = vc[col:col + nf * 128, :].rearrange(
                                        "(c p) f -> p c f", p=128)
                                else:
                                    dst = vt[:, cbase:cbase + DV]
                                    src = vc[col:col + 128, :]
                                veng.dma_start(dst, src)
                                adv = nf * 128
                            else:
                                seg = min(128 - p0, ln)
                                veng.dma_start(
                                    vt[p0:p0 + seg, cbase:cbase + DV],
                                    vc[col:col + seg, :],
                                )
                                adv = seg
                            o += adv
                            col += adv
                            ln -= adv
                qk = qkpool.tile([128, ncp * 2 * G], F32)
                for c in range(ncp):
                    nc.tensor.matmul(
                        qk[:, c * 2 * G:(c + 1) * 2 * G],
                        ktile[:, c * 128:(c + 1) * 128],
                        qtz_sb[:, pi * 2 * G:(pi + 1) * 2 * G],
                        start=True, stop=True, skip_group_check=True,
                    )
                e = epool.tile([128, ncp * 2 * G], F16)
                nc.scalar.activation(
                    e[:], qk[:], mybir.ActivationFunctionType.Exp,
                    scale=1.0 / 8.0,
                )
                pv = pvpool.tile([2 * G, 2 * DV], F32)
                for c in range(ncp):
                    nc.tensor.matmul(
                        pv[:],
                        e[:, c * 2 * G:(c + 1) * 2 * G],
                        vt[:, c * 2 * DV:(c + 1) * 2 * DV],
                        start=(c == 0), stop=(c == ncp - 1),
                    )

                # compute engines need 32-aligned partition starts, so copy
                # pv to SBUF and DMA-shift batch B's quadrant to partition 0
                pvsb = spool.tile([2 * G, 2 * DV], F32, tag="pvsb")
                nc.vector.tensor_copy(pvsb[:], pv[:])
                pvb = spool.tile([G, DV], F32, tag="pvb")
                nc.sync.dma_start(pvb[:], pvsb[G:2 * G, DV:2 * DV])
                for half, b in ((0, bA), (1, bB)):
                    pvh = pvsb[0:G, 0:DV] if half == 0 else pvb[:]
                    den = spool.tile([G, 1], F32, tag="den")
                    nc.vector.tensor_add(den[:], pvh[:, D:DV], esink[:])
                    inv = spool.tile([G, 1], F32, tag="inv")
                    nc.vector.reciprocal(inv[:], den[:])
                    osb = opool.tile([G, D], F32)
                    nc.scalar.mul(osb[:], pvh[:, 0:D], inv[:])
                    nc.sync.dma_start(out[b], osb[:])

    nc.finalize()
    return nc


_PROGRAM_CACHE = {}


def kernel(q_last, k_last, v_last, cache_state, seq_block_ids, start_positions,
           sink):
    global LAST_EXEC_NS

    q_last = np.asarray(q_last, dtype=np.float32)
    k_last = np.asarray(k_last, dtype=np.float32)
    v_last = np.asarray(v_last, dtype=np.float32)
    cache_state = np.asarray(cache_state, dtype=np.float32)
    sbi = np.asarray(seq_block_ids)
    pos = np.asarray(start_positions)
    sink = np.asarray(sink, dtype=np.float32)

    win = [_window_runs(sbi[b], int(pos[b])) for b in range(B)]

    key = (tuple(int(p) for p in pos), sbi.tobytes())
    if key not in _PROGRAM_CACHE:
        _PROGRAM_CACHE.clear()
        _PROGRAM_CACHE[key] = _build_program(win)
    nc = _PROGRAM_CACHE[key]

    pairs = _pairs_of(win)

    # decode-token scatter targets (page-linear)
    scat = [int(sbi[b, int(pos[b]) // S]) * S + int(pos[b]) % S for b in range(B)]

    in_maps = []
    for h in range(Hk):
        kt = np.ascontiguousarray(
            cache_state[:, 0, h].reshape(PL, D).T.astype(F16_NP))
        vcv = np.empty((PL, DV), dtype=F16_NP)
        vcv[:, :D] = cache_state[:, 1, h].reshape(PL, D).astype(F16_NP)
        vcv[:, D] = 1.0
        for b in range(B):
            kt[:, scat[b]] = k_last[b, 0, h].astype(F16_NP)
            vcv[scat[b], :D] = v_last[b, 0, h].astype(F16_NP)
        qz = np.zeros((128, B * G), dtype=F16_NP)
        for pi, (bA, bB) in enumerate(pairs):
            qz[0:D, pi * 2 * G:pi * 2 * G + G] = \
                q_last[bA, 0, h * G:(h + 1) * G].T
            qz[D:128, pi * 2 * G + G:(pi + 1) * 2 * G] = \
                q_last[bB, 0, h * G:(h + 1) * G].T
        skv = np.ascontiguousarray(sink[h * G:(h + 1) * G].reshape(G, 1))
        in_maps.append({"ktc": kt, "vc": vcv, "qtz": qz, "sk": skv})

    prof = os.environ.get("BASS_KERNEL_PROF", "") == "1"
    kwargs = {}
    if prof:
        kwargs = dict(trace=True, trace_cores=list(range(NCORES)))
        tdir = os.environ.get("BASS_KERNEL_PROF_DIR")
        if tdir:
            kwargs["tmpdir"] = tdir
    res = run_bass_kernel_spmd(nc, in_maps, list(range(NCORES)), **kwargs)
    if prof:
        LAST_EXEC_NS = res.exec_time_ns

    full = np.empty((B, 1, Hq, D), dtype=np.float32)
    for h in range(Hk):
        full[:, 0, h * G:(h + 1) * G, :] = res.results[h]["out"]
    return full


# revision 14
# speedup vs baseline: 1.4031x; 1.0561x over previous
"""Bass/Trainium2 kernel for nn_DecodeWrapperEager (sparse paged-attention decode).

Problem (hardcoded): B=8, Hq=32, Hk=8, D=64, S=32, NB=128, T=4096, W=1024.
One decode step of GQA attention with a paged KV cache, sliding window W and
a sink logit.  Output: [B, 1, Hq, D] float32.

Sharding: KV heads across the 8 cores (tensor-parallel).  Each core receives
its head's cache in two layouts staged on host (bf16 for matmul speed):
  - ktc [64, 32768]  : K^T, column = page*32 + slot (page-linear), k_last
                       scattered at the decode position.
  - vc  [32768, 65]  : V rows page-linear, with a fused ones-column (col 64)
                       so the P @ [V|1] matmul yields the softmax denominator.
Per-batch sliding-window offsets are identical on every core, so they are
baked into the single SPMD program; the device gathers only the window pages.
Batches are packed in pairs onto the 128-partition contraction dim (head dim
is 64) so the PE array and all 16 DMA ports are fully used.  Out-of-window
tail positions are handled by zero-filling K columns (score 0 -> weight 1)
and V rows (weight * 0 = 0 contribution), so every matmul is full-size.
"""

import os
import numpy as np

import concourse.bacc as bacc
import concourse.tile as tile
import concourse.mybir as mybir
from concourse.bass_utils import run_bass_kernel_spmd

F32 = mybir.dt.float32
F16 = mybir.dt.float16     # 10-bit mantissa: ~5e-4 end-to-end vs 3e-3 for bf16,
F16_NP = np.float16        # same PE speed (1 cycle/row) and DMA bytes as bf16

B, Hq, Hk, D = 8, 32, 8, 64
S, NB = 32, 128
T = NB * S          # 4096 positions per sequence
P = B * NB          # 1024 cache pages
W = 1024            # sliding window
G = Hq // Hk        # 4 query heads per kv head
NCORES = 8
PL = P * S          # 32768 page-linear positions
DV = D + 1          # V row with fused ones-column

LAST_EXEC_NS = None


def _window_runs(sbi_b, pos):
    """Contiguous runs (col, len) in page-linear space covering [lo, pos]."""
    lo = max(0, pos - W + 1)
    nv = pos + 1 - lo
    runs = []
    t = lo
    while t <= pos:
        blk = t // S
        s0 = t % S
        s1 = min(S - 1, pos - blk * S)
        col = int(sbi_b[blk]) * S + s0
        ln = s1 - s0 + 1
        if runs and runs[-1][0] + runs[-1][1] == col:
            runs[-1][1] += ln
        else:
            runs.append([col, ln])
        t = blk * S + s1 + 1
    return lo, nv, [(c, l) for c, l in runs]


def _pairs_of(win):
    order = sorted(range(B), key=lambda b: -win[b][1])
    return [(order[i], order[i + 1]) for i in range(0, B, 2)]


def _build_program(win):
    """win: list per batch of (lo, nv, runs). Returns finalized Bacc program.

    The program is shared by all 8 cores (SPMD); only the data differs.
    """
    nc = bacc.Bacc("TRN2", debug=False)

    ktc = nc.dram_tensor("ktc", [D, PL], F16, kind="ExternalInput")
    vc = nc.dram_tensor("vc", [PL, DV], F16, kind="ExternalInput")
    qtz = nc.dram_tensor("qtz", [128, B * G], F16, kind="ExternalInput")
    sk = nc.dram_tensor("sk", [G, 1], F32, kind="ExternalInput")
    out = nc.dram_tensor("out", [B, G, D], F32, kind="ExternalOutput")

    pairs = _pairs_of(win)
    nch = {b: (win[b][1] + 127) // 128 for b in range(B)}
    rem = {b: win[b][1] - (nch[b] - 1) * 128 for b in range(B)}

    # V DMAs alternate between otherwise-idle queues to parallelize
    # descriptor generation; K/out/consts go on the sync HWDGE ring.
    vdma_engines = [None, None]

    with tile.TileContext(nc) as tc:
        vdma_engines = [nc.gpsimd, nc.gpsimd]
        with (
            tc.tile_pool(name="const", bufs=1) as cpool,
            tc.tile_pool(name="kt", bufs=3) as kpool,
            tc.tile_pool(name="vv", bufs=3) as vpool,
            tc.tile_pool(name="ee", bufs=3) as epool,
            tc.tile_pool(name="small", bufs=4) as spool,
            tc.tile_pool(name="outp", bufs=4) as opool,
            tc.tile_pool(name="qkp", bufs=3, space="PSUM") as qkpool,
            tc.tile_pool(name="pvp", bufs=3, space="PSUM") as pvpool,
        ):
            qtz_sb = cpool.tile([128, B * G], F16)
            nc.sync.dma_start(qtz_sb[:], qtz[:])
            sk_sb = cpool.tile([G, 1], F32)
            nc.sync.dma_start(sk_sb[:], sk[:])
            esink = cpool.tile([G, 1], F32)
            nc.scalar.activation(esink[:], sk_sb[:], mybir.ActivationFunctionType.Exp)

            for pi, (bA, bB) in enumerate(pairs):
                ncp = max(nch[bA], nch[bB])
                ktile = kpool.tile([128, ncp * 128], F16)
                # extra DV columns of slack so the strided [p, c, 0:DV]
                # multi-chunk DMA view for half 1 stays in bounds
                vt = vpool.tile([128, ncp * 2 * DV + DV], F16)
                for half, b in ((0, bA), (1, bB)):
                    lo, nv, runs = win[b]
                    veng = vdma_engines[half]
                    # K^T window -> rows [half*64, half*64+64)
                    o = 0
                    for col, ln in runs:
                        nc.sync.dma_start(
                            ktile[half * 64:half * 64 + 64, o:o + ln],
                            ktc[:, col:col + ln],
                        )
                        o += ln
                    if nv < ncp * 128:
                        # zero-fill stale K cols: score 0 -> e = 1, killed by
                        # the zero-filled V rows below
                        nc.vector.memset(
                            ktile[half * 64:half * 64 + 64, nv:ncp * 128], 0.0
                        )
                    # zero-fill V beyond the window first (memset needs
                    # 32-aligned partition starts, so cover all 128 rows and
                    # let the window DMAs below overwrite the valid rows)
                    if rem[b] < 128:
                        cbase = (nch[b] - 1) * 2 * DV + half * DV
                        nc.vector.memset(vt[:, cbase:cbase + DV], 0.0)
                    for c in range(nch[b], ncp):
                        cbase = c * 2 * DV + half * DV
                        nc.vector.memset(vt[:, cbase:cbase + DV], 0.0)
                    # V window chunks -> cols [c*130 + half*65, +65)
                    o = 0
                    for col, ln in runs:
                        while ln > 0:
                            c, p0 = divmod(o, 128)
                            cbase = c * 2 * DV + half * DV
                            if p0 == 0 and ln >= 128:
                                nf = ln // 128
                                if nf > 1:
                                    # [p, c, 0:65] strided view over chunk slots
                                    dst = vt[:, cbase:cbase + nf * 2 * DV]
                                    dst = dst.rearrange(
                                        "p (c f) -> p c f", f=2 * DV)[:, :, 0:DV]
                                    src = vc[col:col + nf * 128, :].rearrange(
                                        "(c p) f -> p c f", p=128)
                                else:
                                    dst = vt[:, cbase:cbase + DV]
                                    src = vc[col:col + 128, :]
                                veng.dma_start(dst, src)
                                adv = nf * 128
                            else:
                                seg = min(128 - p0, ln)
                                veng.dma_start(
                                    vt[p0:p0 + seg, cbase:cbase + DV],
                                    vc[col:col + seg, :],
                                )
                                adv = seg
                            o += adv
                            col += adv
                            ln -= adv
                qk = qkpool.tile([128, ncp * 2 * G], F32)
                for c in range(ncp):
                    nc.tensor.matmul(
                        qk[:, c * 2 * G:(c + 1) * 2 * G],
                        ktile[:, c * 128:(c + 1) * 128],
                        qtz_sb[:, pi * 2 * G:(pi + 1) * 2 * G],
                        start=True, stop=True, skip_group_check=True,
                    )
                e = epool.tile([128, ncp * 2 * G], F16)
                nc.scalar.activation(
                    e[:], qk[:], mybir.ActivationFunctionType.Exp,
                    scale=1.0 / 8.0,
                )
                pv = pvpool.tile([2 * G, 2 * DV], F32)
                for c in range(ncp):
                    nc.tensor.matmul(
                        pv[:],
                        e[:, c * 2 * G:(c + 1) * 2 * G],
                        vt[:, c * 2 * DV:(c + 1) * 2 * DV],
                        start=(c == 0), stop=(c == ncp - 1),
                    )

                # compute engines need 32-aligned partition starts, so copy
                # pv to SBUF and DMA-shift batch B's quadrant to partition 0
                pvsb = spool.tile([2 * G, 2 * DV], F32, tag="pvsb")
                nc.vector.tensor_copy(pvsb[:], pv[:])
                pvb = spool.tile([G, DV], F32, tag="pvb")
                nc.sync.dma_start(pvb[:], pvsb[G:2 * G, DV:2 * DV])
                for half, b in ((0, bA), (1, bB)):
                    pvh = pvsb[0:G, 0:DV] if half == 0 else pvb[:]
                    den = spool.tile([G, 1], F32, tag="den")
                    nc.vector.tensor_add(den[:], pvh[:, D:DV], esink[:])
                    inv = spool.tile([G, 1], F32, tag="inv")
                    nc.vector.reciprocal(inv[:], den[:])
                    osb = opool.tile([G, D], F32)
                    nc.scalar.mul(osb[:], pvh[:, 0:D], inv[:])
                    nc.sync.dma_start(out[b], osb[:])

    nc.finalize()
    return nc


_PROGRAM_CACHE = {}


def kernel(q_last, k_last, v_last, cache_state, seq_block_ids, start_positions,
           sink):
    global LAST_EXEC_NS

    q_last = np.asarray(q_last, dtype=np.float32)
    k_last = np.asarray(k_last, dtype=np.float32)
    v_last = np.asarray(v_last, dtype=np.float32)
    cache_state = np.asarray(cache_state, dtype=np.float32)
    sbi = np.asarray(seq_block_ids)
    pos = np.asarray(start_positions)
    sink = np.asarray(sink, dtype=np.float32)

    win = [_window_runs(sbi[b], int(pos[b])) for b in range(B)]

    key = (tuple(int(p) for p in pos), sbi.tobytes())
    if key not in _PROGRAM_CACHE:
        _PROGRAM_CACHE.clear()
        _PROGRAM_CACHE[key] = _build_program(win)
    nc = _PROGRAM_CACHE[key]

    pairs = _pairs_of(win)

    # decode-token scatter targets (page-linear)
    scat = [int(sbi[b, int(pos[b]) // S]) * S + int(pos[b]) % S for b in range(B)]

    in_maps = []
    for h in range(Hk):
        kt = np.ascontiguousarray(
            cache_state[:, 0, h].reshape(PL, D).T.astype(F16_NP))
        vcv = np.empty((PL, DV), dtype=F16_NP)
        vcv[:, :D] = cache_state[:, 1, h].reshape(PL, D).astype(F16_NP)
        vcv[:, D] = 1.0
        for b in range(B):
            kt[:, scat[b]] = k_last[b, 0, h].astype(F16_NP)
            vcv[scat[b], :D] = v_last[b, 0, h].astype(F16_NP)
        qz = np.zeros((128, B * G), dtype=F16_NP)
        for pi, (bA, bB) in enumerate(pairs):
            qz[0:D, pi * 2 * G:pi * 2 * G + G] = \
                q_last[bA, 0, h * G:(h + 1) * G].T
            qz[D:128, pi * 2 * G + G:(pi + 1) * 2 * G] = \
                q_last[bB, 0, h * G:(h + 1) * G].T
        skv = np.ascontiguousarray(sink[h * G:(h + 1) * G].reshape(G, 1))
        in_maps.append({"ktc": kt, "vc": vcv, "qtz": qz, "sk": skv})

    prof = os.environ.get("BASS_KERNEL_PROF", "") == "1"
    kwargs = {}
    if prof:
        kwargs = dict(trace=True, trace_cores=list(range(NCORES)))
        tdir = os.environ.get("BASS_KERNEL_PROF_DIR")
        if tdir:
            kwargs["tmpdir"] = tdir
    res = run_bass_kernel_spmd(nc, in_maps, list(range(NCORES)), **kwargs)
    if prof:
        LAST_EXEC_NS = res.exec_time_ns

    full = np.empty((B, 1, Hq, D), dtype=np.float32)
    for h in range(Hk):
        full[:, 0, h * G:(h + 1) * G, :] = res.results[h]["out"]
    return full
